# revision 20
# baseline (speedup 1.0000x reference)
"""GaussianImage rasterization kernel for Trainium2 (8 NeuronCores).

Math: out(h,w,c) = rgb[-1,c]*alpha[-1] * S(h,w),
      S = sum_n exp(-0.5 (p-m_n)^T InvCov_n (p-m_n))

The exponent is expanded into a 6-feature dot product per (gaussian, pixel):
  expo = g1*(4x'^2) + g2*(4x'y') + g3*(4y'^2) + g4*(2x') + g5*(2y') + g6
with x' = x-0.5, y' = y-0.5 (centering improves conditioning).

Each core rasterizes 64 image rows (32768 pixels) against all 128 gaussians:
  - 256 pixel-blocks of 128 pixels; block j holds pixels {q*256+j, q=0..127}
  - matmul: lhsT = fp16 feature rows (K=17, M=128 px), rhs = fp16 coeff rows
    (K=17, N=128 gaussians) -> PSUM (128 px, 128 gauss per block) fp32
  - fp16 hi/mid/lo 3-way splitting of operands gives ~fp32-accurate
    products (32 paired rows; K is time-free on the PE, only N matters)
  - ScalarE: exp over PSUM -> fp16 SBUF (the saturated engine, ~2us/round)
  - VectorE: two halving adds + tensor_reduce -> per-pixel sums (fp16),
    software-pipelined one round behind the ScalarE exp
  - channel scaling on the host; device ships fp16 S only (64KB/core)
"""

import numpy as np

N_GAUSS = 128
H = 512
W = 512
N_CORES = 8
ROWS_PER_CORE = H // N_CORES          # 64
PX_PER_CORE = ROWS_PER_CORE * W       # 32768
N_BLOCKS = PX_PER_CORE // 128         # 256 blocks of 128 px
K_ROWS = 32                           # fp16 3-way split pairs

# Schraudolph fp16 exp: bits16(exp(x)) ~ int16(A16*x + B16).  B16/A16 is
# folded into the constant coefficient (g6 += SHIFT), so PSUM holds
# expo' = expo + SHIFT.  ScalarE computes exp(expo' - SHIFT) exactly;
# VectorE computes int16(max(expo',0)*A16) (bias 52.5 tuned mean-unbiased).
A16 = 1024.0 / np.log(2.0)
SHIFT = (15.0 * 1024.0 - 52.5) / A16

# round sizes in blocks: small first rounds start the scalar engine earlier
ROUND_BLOCKS = [4, 4, 8] + [16] * 14 + [8, 4, 4]
assert sum(ROUND_BLOCKS) == N_BLOCKS
# rounds whose first half is computed via the DVE bit-trick exp, sized to
# the vector engine's aggregate slack; placed on the LAST 16-block rounds so
# the delayed PSUM release cannot stall later matmuls
SCHRAUD_ROUNDS = set()

# feature DMA chunks (in pixel columns); first small for fast pipeline start
FEAT_CHUNKS = [512, 1536, 2048, 4096, 8192, 16384]
FEAT_OFFS = [0]
for _w in FEAT_CHUNKS:
    FEAT_OFFS.append(FEAT_OFFS[-1] + _w)
assert FEAT_OFFS[-1] == PX_PER_CORE

# ---------------------------------------------------------------------------
# Host-side math (fp64): coefficients, features, fp16 splitting
# ---------------------------------------------------------------------------

def _f16_split3(v):
    """Split fp64 array into fp16 hi, mid, lo with v ~ hi+mid+lo."""
    hi = v.astype(np.float16)
    r1 = v - hi.astype(np.float64)
    mid = r1.astype(np.float16)
    r2 = r1 - mid.astype(np.float64)
    lo = r2.astype(np.float16)
    return hi, mid, lo


def _coeffs(mean, scale, theta):
    """Per-gaussian coefficients g1..g6 (fp64), feature-scaled."""
    m = mean.astype(np.float64)
    s = scale.astype(np.float64)
    th = (1.0 + np.sin(theta.astype(np.float64)[:, 0])) * np.pi
    c, sn = np.cos(th), np.sin(th)
    is1 = 1.0 / s[:, 0] ** 2
    is2 = 1.0 / s[:, 1] ** 2
    A = c * c * is1 + sn * sn * is2
    B = c * sn * (is1 - is2)
    C = sn * sn * is1 + c * c * is2
    mx = m[:, 0] - 0.5
    my = m[:, 1] - 0.5
    # features are [4x'^2, 4x'y', 4y'^2, 2x', 2y', 1]
    g = np.stack([
        -A / 8.0,
        -B / 4.0,
        -C / 8.0,
        (A * mx + B * my) / 2.0,
        (B * mx + C * my) / 2.0,
        -0.5 * (A * mx * mx + 2.0 * B * mx * my + C * my * my) + SHIFT,
    ], axis=0)  # (6, N)
    return g


def _features(pixels_flat):
    """Feature rows (6, P) fp64 from pixel coords (P, 2)."""
    p = pixels_flat.astype(np.float64)
    x = p[:, 0] - 0.5
    y = p[:, 1] - 0.5
    return np.stack([4*x*x, 4*x*y, 4*y*y, 2*x, 2*y, np.ones_like(x)], axis=0)


# Paired rows: (feature_index, f_piece, g_piece); pieces: 0=hi 1=mid 2=lo.
# 32 rows: the mm/hl/lh tail is below the fp16 exp-output rounding floor but
# keeping 32 DMA rows preserves even packet fan-out across the DMA engines.
def _row_plan():
    plan = []
    big = [2, 1, 4, 0, 3]  # y^2, xy, y, x^2, x  (largest |g*f| first)
    for f in big:
        plan.append((f, 0, 0))   # hh
    plan.append((5, 0, 0))       # const * g_hi
    plan.append((5, 0, 1))       # const * g_mid
    for f in big:
        plan.append((f, 0, 1))   # hm
        plan.append((f, 1, 0))   # mh
    for f in big:
        plan.append((f, 1, 1))   # mm
        plan.append((f, 0, 2))   # hl
        plan.append((f, 2, 0))   # lh
    assert len(plan) == K_ROWS
    return plan


def _host_prep(mean, rgb, alpha, scale, theta, pixels):
    """Build per-core device operands."""
    plan = _row_plan()
    g = _coeffs(mean, scale, theta)              # (6, 128) fp64
    g_pieces = [_f16_split3(g[f]) for f in range(6)]   # list of (hi,mid,lo)

    coef = np.stack([g_pieces[f][gp] for (f, _fp, gp) in plan],
                    axis=0).astype(np.float16)   # (32, 128)

    rgba = (rgb[-1].astype(np.float64) * alpha[-1, 0].astype(np.float64))

    # Pixel-block layout: within a core's 32768 pixels (p = q*256 + j),
    # block j holds pixels {q*256+j : q}.  F_sb[k, j*128+q] = F32[k, q*256+j].
    pix = np.asarray(pixels).reshape(H * W, 2)
    feats = []
    for core in range(N_CORES):
        pf = pix[core * PX_PER_CORE:(core + 1) * PX_PER_CORE]
        F = _features(pf)                        # (6, 32768) fp64
        f_pieces = [_f16_split3(F[f]) for f in range(6)]
        F32 = np.stack([f_pieces[f][fp] for (f, fp, _gp) in plan], axis=0)
        Fb = F32.reshape(K_ROWS, 128, 256)       # [k, q, j]
        Fb = Fb.transpose(0, 2, 1)               # [k, j, q]
        Fsb = Fb.reshape(K_ROWS, 256 * 128)      # partition k, col j*128+q
        feats.append(np.ascontiguousarray(Fsb.astype(np.float16)))
    return feats, coef, rgba


# ---------------------------------------------------------------------------
# Device kernel
# ---------------------------------------------------------------------------

_CACHE = {}


def _build_bass():
    import concourse.bacc as bacc
    import concourse.mybir as mybir
    from concourse.tile import TileContext

    fp16 = mybir.dt.float16
    i16 = mybir.dt.int16
    f32 = mybir.dt.float32

    nc = bacc.Bacc("TRN2", target_bir_lowering=False)
    feat_d = [
        nc.dram_tensor(f"feat{t}", [K_ROWS, w], fp16, kind="ExternalInput")
        for t, w in enumerate(FEAT_CHUNKS)
    ]
    coef_d = nc.dram_tensor("coef", [K_ROWS, 128], fp16, kind="ExternalInput")
    out_d = nc.dram_tensor("out", [128, 256], fp16, kind="ExternalOutput")

    with TileContext(nc) as tc:
        with (
            tc.tile_pool(name="const", bufs=1) as cpool,
            tc.tile_pool(name="feat", bufs=1) as fpool,
            tc.tile_pool(name="psum", bufs=2, space="PSUM") as ppool,
            tc.tile_pool(name="splat", bufs=5) as spool,
            tc.tile_pool(name="scratch", bufs=4) as scpool,
            tc.tile_pool(name="acc", bufs=1) as apool,
        ):
            # warm the exp table while DMAs stream; bias tile for -SHIFT
            dummy = cpool.tile([128, 1], fp16, tag="dummy")
            nc.gpsimd.memset(dummy[:], 0)
            bias_t = cpool.tile([128, 1], f32, tag="bias")
            nc.gpsimd.memset(bias_t[:], -SHIFT)
            nc.scalar.activation(dummy[:], dummy[:],
                                 mybir.ActivationFunctionType.Exp)

            g_sb = cpool.tile([K_ROWS, 128], fp16, tag="gsb")
            nc.scalar.dma_start(g_sb[:], coef_d[:])

            ftiles = []
            for t, fd in enumerate(feat_d):
                ft = fpool.tile(list(fd.shape), fp16, tag=f"ft{t}")
                ftiles.append(ft)
            nc.sync.dma_start(ftiles[0][:], feat_d[0][:])
            # chunk1 rides the scalar queue so its descriptor generation
            # overlaps with chunk0's on the sync queue
            nc.scalar.dma_start(ftiles[1][:], feat_d[1][:])
            for t in range(2, len(feat_d)):
                nc.sync.dma_start(ftiles[t][:], feat_d[t][:])

            S_big = apool.tile([128, 256], fp16, tag="sbig")

            def reduce_chain(sp, nb, blk0):
                """pair-adds within each 128-gauss block, then reduce."""
                w = nb * 128
                sp3 = sp[:, :w].rearrange("p (i g) -> p i g", g=128)
                sc = scpool.tile([128, 1024], fp16, tag="sc")
                sc3 = sc[:, :nb * 64].rearrange("p (i g) -> p i g", g=64)
                nc.vector.tensor_tensor(
                    sc3, sp3[:, :, 0:64], sp3[:, :, 64:128],
                    op=mybir.AluOpType.add,
                )
                sc3b = sc[:, :nb * 32].rearrange("p (i g) -> p i g", g=32)
                nc.vector.tensor_tensor(
                    sc3b, sc3[:, :, 0:32], sc3[:, :, 32:64],
                    op=mybir.AluOpType.add,
                )
                with nc.allow_low_precision(reason="S~30, fp16 ulp 0.016"):
                    nc.vector.tensor_reduce(
                        S_big[:, blk0:blk0 + nb], sc3b,
                        axis=mybir.AxisListType.X, op=mybir.AluOpType.add,
                    )
                # stream the output out in quarters to overlap the tail
                end = blk0 + nb
                if end % 64 == 0:
                    q = end // 64 - 1
                    nc.sync.dma_start(out_d[:, 64 * q:64 * (q + 1)],
                                      S_big[:, 64 * q:64 * (q + 1)])

            # software pipeline: round r's reduce chain is emitted during
            # round r+1, so the DVE backlog never delays PSUM release
            blk = 0  # global block index
            pending = None
            for r, nb in enumerate(ROUND_BLOCKS):
                w = nb * 128
                cw = w // 2 if r in SCHRAUD_ROUNDS else 0
                ps = ppool.tile([128, 2048], f32, tag="ps")
                for i in range(nb):
                    gcol = (blk + i) * 128
                    t = next(c for c in range(len(FEAT_CHUNKS))
                             if FEAT_OFFS[c + 1] > gcol)
                    off = gcol - FEAT_OFFS[t]
                    nc.tensor.matmul(
                        ps[:, i * 128:(i + 1) * 128],
                        ftiles[t][:, off:off + 128], g_sb[:],
                    )
                sp = spool.tile([128, 2048], fp16, tag="sp")
                # ScalarE: true exp (contiguous)
                nc.scalar.activation(sp[:, cw:w], ps[:, cw:w],
                                     mybir.ActivationFunctionType.Exp,
                                     bias=bias_t[:])
                if cw:
                    # VectorE: Schraudolph bit-trick exp on the first half
                    nc.vector.tensor_scalar(
                        sp[:, :cw].bitcast(i16),
                        ps[:, :cw], 0.0, A16,
                        op0=mybir.AluOpType.max, op1=mybir.AluOpType.mult,
                    )
                if pending is not None:
                    reduce_chain(*pending)
                pending = (sp, nb, blk)
                blk += nb
            reduce_chain(*pending)

    nc.finalize()
    return nc


def _run(inputs, trace=False):
    from concourse.bass_utils import run_bass_kernel_spmd

    feats, coef, rgba = _host_prep(**inputs)
    if "nc" not in _CACHE:
        _CACHE["nc"] = _build_bass()
    nc = _CACHE["nc"]

    in_maps = []
    for core in range(N_CORES):
        fc = feats[core]
        m = {f"feat{t}": np.ascontiguousarray(
                fc[:, FEAT_OFFS[t]:FEAT_OFFS[t + 1]])
             for t in range(len(FEAT_CHUNKS))}
        m["coef"] = coef
        in_maps.append(m)

    res = run_bass_kernel_spmd(
        nc, in_maps, core_ids=list(range(N_CORES)), trace=trace,
    )
    shards = []
    for core in range(N_CORES):
        S = res.results[core]["out"].astype(np.float32)   # (128, 256)
        shards.append(S.reshape(64, 512))
    S_full = np.concatenate(shards, axis=0)               # (512, 512)
    out = S_full[:, :, None] * rgba.astype(np.float32)[None, None, :]
    return np.ascontiguousarray(out, dtype=np.float32), res


def kernel(mean, rgb, alpha, scale, theta, pixels):
    out, _ = _run(dict(mean=mean, rgb=rgb, alpha=alpha, scale=scale,
                       theta=theta, pixels=pixels))
    return out
